# revision 1
# baseline (speedup 1.0000x reference)
"""Single-head causal attention (B=8, T=2048, C=1024, H=128) on 8 TRN2 NeuronCores.

Strategy: pure data-parallel over batch — one batch element per core, zero
collectives.  Per core, everything is laid out so every matmul contracts on
the partition dimension and softmax normalization folds into the PV matmul:

  - host relayouts X[b] -> [half, pair, p, chunk-in-pair, 1024] and
    W -> [P, C/P, 3, H] (both bf16) so every input DMA is 4KB/partition
    contiguous (full per-queue DMA rate); pieces are issued in demand order
    across the gpsimd/scalar/sync queues so (w3, first xt pair) land first
    and projection half 0 never starves.
  - Q^T, K^T, V^T  [H, T] = W.T @ XT  (lhsT = W chunk, rhs = XT chunk),
    accumulated per 1024-wide t-half (6 PSUM banks), drained with bias add +
    bf16 cast split between VectorE and ScalarE so the first S matmuls of
    the following block start as soon as their q/k segments exist.
  - V^T is transposed on the PE (16 128x128 blocks) into V tiles [T, H+1],
    with a ones column appended so the PV matmul also produces the softmax
    denominator.
  - S^T pair-blocks [k(2x128), q(512)] into one [128,1024] PSUM tile; ONE
    exp(S^T/sqrt(C)) on ScalarE per pair (no max subtraction; logits are
    small; exp of never-read uncausal blocks is bounded).  Causal triangle =
    [128,128] mask multiplies on the diagonal blocks, on GpSimd.
  - out_aug [q(128), H+1] accumulates ES-block.T @ V_aug over k-tiles into
    slot-packed 1-bank PSUM tiles (4 chains in flight in 2 banks); last
    column is the softmax denominator; VectorE reciprocal + scale; one
    output DMA per 128-row q tile, rotated across queues (the final chains
    use the idle scalar queue so the end-of-kernel drain has no backlog).
  - emission order: proj0, S0, vtrans, S1, PV0, proj1, then a weave that
    puts S3's pairs first (their exps are the long ScalarE tail), fills the
    exp-pacing stalls with PV1 chains + vtrans, then PV3 woven with S2
    pairs, and PV2 last (engine queues execute in order, so emission order
    IS the schedule).
"""

import numpy as np
import ml_dtypes

import concourse.bass as bass  # noqa: F401
import concourse.mybir as mybir
import concourse.tile as tile
from concourse import bacc
from concourse.bass_utils import run_bass_kernel_spmd

B, T, C, H = 8, 2048, 1024, 128
NCORES = 8
P = 128
SEG = 512
BF16 = mybir.dt.bfloat16
F32 = mybir.dt.float32
SCALE = float(C) ** -0.5

LAST_RESULT = None


def build_nc(t=T, reps=1):
    nchunk = C // P      # 8
    ntile = t // P       # t tiles
    nblk = t // SEG      # q blocks
    tpb = SEG // P       # 4
    nhalf = max(1, t // 1024)
    hw = t // nhalf      # 1024 at full size

    nc = bacc.Bacc("TRN2", target_bir_lowering=False, debug=False)

    npair = (C // P) // 2
    xt_d = nc.dram_tensor("xt", [2, npair, P, 2, t // 2], BF16,
                          kind="ExternalInput")
    w3_d = nc.dram_tensor("w3", [P, nchunk, 3, H], BF16, kind="ExternalInput")
    b3_d = nc.dram_tensor("b3", [H, 3], F32, kind="ExternalInput")
    te_d = nc.dram_tensor("te", [P, 2 * P], BF16, kind="ExternalInput")
    out_d = nc.dram_tensor("out", [t, H], F32, kind="ExternalOutput")

    Exp = mybir.ActivationFunctionType.Exp
    Ident = mybir.ActivationFunctionType.Identity

    with tile.TileContext(nc) as tc:
        with (
            tc.tile_pool(name="const", bufs=1) as cpool,
            tc.tile_pool(name="big", bufs=1) as bpool,
            tc.tile_pool(name="v", bufs=ntile) as vpool,
            tc.tile_pool(name="es", bufs=max(2, (ntile // 4 + 1) * ntile // 4)) as espool,
            tc.tile_pool(name="o", bufs=1) as opool,
            tc.tile_pool(name="ps", bufs=1, space="PSUM") as pspool,
        ):
          for rep in range(reps):
            # ---- input DMAs: strictly ordered so (w3[0], xt[0]) land first.
            # HBM is the startup constraint (all 8 cores pull ~5MB each at
            # once); serial per-queue order = priority, and the projection
            # consumes chunk cc slower than the DMA supplies chunk cc+1.
            # xt pair-chunk tiles [P, 2(chunk-in-pair), t]: the host lays
            # out each (pair, half) block so every DMA is 4KB/partition
            # contiguous (full DMA-queue rate) while halves stay separately
            # schedulable: first halves land before projection half 0 needs
            # them, second halves queue behind on the same engines.
            # one tile per (pair, half): the projection's half-0 matmuls
            # must not depend on the half-1 DMAs (tile-granular deps)
            xt_s = {(pp, hh): cpool.tile([P, 2, t // 2], BF16,
                                         tag=f"xtp{pp}_{hh}",
                                         name=f"xtp{pp}_{hh}")
                    for pp in range(npair) for hh in range(2)}
            w_s = cpool.tile([P, nchunk, 3, H], BF16, tag="w3", name="w3_t")
            nc.sync.dma_start(out=w_s[:, 0:nchunk // 2],
                              in_=w3_d[:, 0:nchunk // 2])
            nc.sync.dma_start(out=w_s[:, nchunk // 2:nchunk],
                              in_=w3_d[:, nchunk // 2:nchunk])
            # each (pair, half) block split by partition across both queues:
            # a pair lands in ~half the time, strictly demand-ordered
            for hh in range(2):
                for pp in range(npair):
                    xs = xt_s[(pp, hh)]
                    nc.gpsimd.dma_start(out=xs[0:P // 2],
                                        in_=xt_d[hh, pp, 0:P // 2])
                    nc.scalar.dma_start(out=xs[P // 2:P],
                                        in_=xt_d[hh, pp, P // 2:P])
            te_s = cpool.tile([P, 2 * P], BF16, tag="te", name="te_t")
            nc.sync.dma_start(out=te_s[:], in_=te_d[:])
            b_s = cpool.tile([P, 3], F32, tag="b3", name="b3_t")
            nc.sync.dma_start(out=b_s[:], in_=b3_d[:])
            tri_s = te_s[:, 0:P]
            eye_s = te_s[:, P:2 * P]

            qt_s = bpool.tile([P, t], BF16, tag="qt", name="qt_t")
            kt_s = bpool.tile([P, t], BF16, tag="kt", name="kt_t")
            vt_s = bpool.tile([P, t], BF16, tag="vt", name="vt_t")
            v_s = [None] * ntile
            W = {"wq": 0, "wk": 1, "wv": 2}

            def proj_half(hf):
                base = hf * hw
                off2 = 0
                acc = {}
                for name in ("wq", "wk", "wv"):
                    acc[name] = pspool.tile([P, hw], F32, tag="ps2", bufs=3,
                                            name=f"ps_{name}_h{hf}")
                for cc in range(nchunk):
                    for name in ("wq", "wk", "wv"):
                        for s2 in range(hw // SEG):
                            nc.tensor.matmul(
                                acc[name][:, s2 * SEG:(s2 + 1) * SEG],
                                w_s[:, cc, W[name], :],
                                xt_s[(cc // 2, hf)][:, cc % 2,
                                              off2 + s2 * SEG: off2 + (s2 + 1) * SEG],
                                start=(cc == 0), stop=(cc == nchunk - 1),
                            )
                # Q/K/V drains split across VectorE+ScalarE so the first S
                # matmul (needs kt tiles 0-3 + qt first 512) starts ASAP
                nc.vector.tensor_scalar_add(
                    kt_s[:, base:base + SEG], acc["wk"][:, 0:SEG], b_s[:, 1:2])
                nc.scalar.activation(
                    qt_s[:, base:base + SEG], acc["wq"][:, 0:SEG], Ident,
                    bias=b_s[:, 0:1])
                nc.vector.tensor_scalar_add(
                    qt_s[:, base + SEG:base + hw], acc["wq"][:, SEG:hw],
                    b_s[:, 0:1])
                nc.scalar.activation(
                    kt_s[:, base + SEG:base + hw], acc["wk"][:, SEG:hw], Ident,
                    bias=b_s[:, 1:2])
                nc.vector.tensor_scalar_add(
                    vt_s[:, base:base + SEG], acc["wv"][:, 0:SEG], b_s[:, 2:3])
                nc.vector.tensor_scalar_add(
                    vt_s[:, base + SEG:base + hw], acc["wv"][:, SEG:hw],
                    b_s[:, 2:3])

            def vtrans(m):
                pst = pspool.tile([P, P], BF16, tag="pso", bufs=2, name=f"pst{m}")
                nc.tensor.transpose(pst[:], vt_s[:, m * P:(m + 1) * P], eye_s)
                v = vpool.tile([P, H + 1], BF16, tag="v", name=f"vtile{m}")
                nc.vector.tensor_copy(v[:, 0:H], pst[:])
                nc.vector.memset(v[:, H:H + 1], 1.0)
                v_s[m] = v

            es_all = {}

            def S_pair(j, p_):
                es_of = es_all.setdefault(j, [None] * (tpb * j + tpb))
                m0 = 2 * p_
                ps2 = pspool.tile([P, 2 * SEG], F32, tag="ps2", bufs=3,
                                  name=f"pss{j}_{p_}")
                es2 = espool.tile([P, 2 * SEG], BF16, tag="es", name=f"es{j}_{p_}")
                diag = m0 >= tpb * j
                for u in range(2):
                    m = m0 + u
                    r = m - tpb * j
                    off = P * r if r > 0 else 0
                    nc.tensor.matmul(
                        ps2[:, SEG * u + off:SEG * (u + 1)],
                        kt_s[:, m * P:(m + 1) * P],
                        qt_s[:, j * SEG + off:(j + 1) * SEG],
                        start=True, stop=True,
                    )
                nc.scalar.activation(es2[:], ps2[:], Exp, scale=SCALE)
                if diag:
                    # exp of the strictly-uncausal blocks is bounded (logits
                    # are small) and never read by PV; only the diagonal
                    # blocks need the triangle mask
                    for u in range(2):
                        m = m0 + u
                        r = m - tpb * j
                        off = P * r if r > 0 else 0
                        nc.gpsimd.tensor_mul(
                            es2[:, SEG * u + off:SEG * u + off + P],
                            es2[:, SEG * u + off:SEG * u + off + P], tri_s)
                es_of[m0] = (es2, 0)
                es_of[m0 + 1] = (es2, 1)

            def att_S(j):
                for p_ in range(tpb * j + tpb):
                    if 2 * p_ < tpb * j + tpb:
                        S_pair(j, p_)

            out_q = [nc.sync, nc.gpsimd, nc.sync, nc.gpsimd]
            # j=2 is emitted last: its final chains' DMAs go to the idle
            # scalar queue so the kernel-end drain isn't stuck behind
            # earlier output transfers on sync/gpsimd
            out_q2 = [nc.gpsimd, nc.sync, nc.scalar, nc.scalar]

            def PV_prep(j):
                ob = opool.tile([P, tpb, H], F32, tag="ob", bufs=2, name=f"ob{j}")
                # two 1-bank PSUM tiles, each holding 2 chain slots: 4
                # in-flight PV chains (PE runs ahead while DVE drains)
                pts = [pspool.tile([P, 2, H + 32], F32, tag="pso", bufs=2,
                                   name=f"psoT{j}_{x}") for x in range(2)]
                return ob, pts

            def PV_chain(j, rr, ob, pts):
                es_of = es_all[j]
                i = tpb * j + rr
                pso = pts[rr % 2][:, rr // 2, 0:H + 1]
                for m in range(i + 1):
                    es2, u = es_of[m]
                    nc.tensor.matmul(
                        pso[:],
                        es2[:, SEG * u + rr * P:SEG * u + rr * P + P],
                        v_s[m][:],
                        start=(m == 0), stop=(m == i),
                    )
                rc = opool.tile([P, 1], F32, tag="rc", bufs=4, name=f"rc{i}")
                nc.vector.reciprocal(rc[:], pso[:, H:H + 1])
                nc.vector.tensor_scalar_mul(ob[:, rr, :], pso[:, 0:H], rc[:])
                q = out_q2[rr] if j == 2 else out_q[i % 4]
                q.dma_start(out=out_d[i * P:(i + 1) * P, :], in_=ob[:, rr, :])

            def att_PV(j):
                ob, pts = PV_prep(j)
                for rr in range(tpb):
                    PV_chain(j, rr, ob, pts)

            if t >= 2048:
                proj_half(0)
                att_S(0)
                for m in range(ntile // 2):
                    vtrans(m)
                att_S(1)
                att_PV(0)
                proj_half(1)
                # back half: S3 pairs first (their exps are the long ACT
                # tail), woven with PV1 chains + vtrans so the PE fills the
                # exp-pacing stalls; then PV3 (gated by S3 exps) woven with
                # S2 pairs; PV2 (gated by S2 exps) last
                ob1, pts1 = PV_prep(1)
                S_pair(3, 0); S_pair(3, 1); S_pair(3, 2)
                PV_chain(1, 0, ob1, pts1)
                S_pair(3, 3)
                PV_chain(1, 1, ob1, pts1)
                S_pair(3, 4)
                PV_chain(1, 2, ob1, pts1)
                S_pair(3, 5)
                PV_chain(1, 3, ob1, pts1)
                S_pair(3, 6)
                for m in range(ntile // 2, ntile // 2 + 4):
                    vtrans(m)
                S_pair(3, 7)
                for m in range(ntile // 2 + 4, ntile):
                    vtrans(m)
                ob3, pts3 = PV_prep(3)
                S_pair(2, 0)
                PV_chain(3, 0, ob3, pts3)
                S_pair(2, 1)
                PV_chain(3, 1, ob3, pts3)
                S_pair(2, 2)
                PV_chain(3, 2, ob3, pts3)
                S_pair(2, 3)
                PV_chain(3, 3, ob3, pts3)
                S_pair(2, 4); S_pair(2, 5)
                att_PV(2)
            else:
                for hf in range(nhalf):
                    proj_half(hf)
                for m in range(ntile):
                    vtrans(m)
                for j in range(nblk):
                    att_S(j)
                    att_PV(j)

    nc.finalize()
    return nc


_NC_CACHE = {}


def _get_nc(t=T, reps=1):
    key = (t, reps)
    if key not in _NC_CACHE:
        _NC_CACHE[key] = build_nc(t, reps)
    return _NC_CACHE[key]


def make_in_maps(embedded_data, Wq, bq, Wk, bk, Wv, bv, t=T):
    bf = ml_dtypes.bfloat16
    tri = np.triu(np.ones((P, P), dtype=np.float32))  # tri[k,q]=1 iff q>=k
    eye = np.eye(P, dtype=np.float32)
    te = np.concatenate([tri, eye], axis=1).astype(bf)
    w3 = np.stack([np.asarray(w, np.float32) for w in (Wq, Wk, Wv)])  # [3,C,H]
    # pre-transpose to [P, 3, C//P, H] so the DMA is contiguous per partition
    w3 = np.ascontiguousarray(
        w3.reshape(3, C // P, P, H).transpose(2, 1, 0, 3)).astype(bf)
    b3 = np.stack(
        [np.asarray(x, np.float32).reshape(H) for x in (bq, bk, bv)], axis=1)
    shared = {"w3": w3, "b3": np.ascontiguousarray(b3), "te": te}
    in_maps = []
    for b in range(NCORES):
        m = dict(shared)
        xtf = np.asarray(embedded_data[b], np.float32).T[:, :t]  # [C, t]
        # [pp, e, p, h, col] -> [h, pp, p, e, col]
        arr = xtf.reshape(C // P // 2, 2, P, 2, t // 2).transpose(3, 0, 2, 1, 4)
        m["xt"] = np.ascontiguousarray(arr).astype(bf)
        in_maps.append(m)
    return in_maps


def kernel(embedded_data, Wq, bq, Wk, bk, Wv, bv, trace=False):
    global LAST_RESULT
    nc = _get_nc(T)
    in_maps = make_in_maps(embedded_data, Wq, bq, Wk, bk, Wv, bv, T)
    res = run_bass_kernel_spmd(nc, in_maps, core_ids=list(range(NCORES)), trace=trace)
    LAST_RESULT = res
    out = np.stack([np.asarray(res.results[i]["out"]) for i in range(NCORES)])
    return out.astype(np.float32)

